# revision 44
# baseline (speedup 1.0000x reference)
"""GCN message-passing kernel for 8 Trainium2 NeuronCores (Bass/Tile).

Computes (matching the jax reference):
    h = x @ W_conv                      [N, H]
    node_embed = leaky_relu(D^-1/2 (A+I) D^-1/2 h + b_conv)
    out = sigmoid(leaky(cat(e[i], e[j]) @ W1 + b1) @ W2 + b2)

Distribution: nodes dst-sharded over the 8 cores. The scaled features
g = dinv * h are AllGathered in TWO chunks (tiles 0-47 / 48-97) so the
second chunk's collective overlaps the first chunk's edge processing;
gather buckets are keyed (AG chunk x src parity) so chunk-A windows
start as soon as AG-A lands. Per-edge source rows are fetched with bulk
SWDGE dma_gather (pair-packed fp16 rows; int16 indices stay in range
because each chunk table has <=25600 pair rows; 4 parallel SWDGE
queues; each window split into two half-gathers so descriptor drain
overlaps generation). Edges are packed into pooled per-(group,bucket)
chunk streams and scatter-added on the TensorEngine via fp8 one-hot
matmuls GENERATED ON-DEVICE (one DVE is_equal against an iota tile per
window) instead of streamed from DRAM. Self-loop contributions are
added locally from the resident g tiles (an all-DVE leaky chain). The
node embeddings are AllGathered with the same 2-chunk overlap: chunk A
fires mid-edge-phase, chunk B right at edge end, and the pair stream is
split into two PSUM accumulation phases (chunk-A windows copied into
xs, chunk-B windows added) so pair chunk-A gathers run underneath the
final e AllGather. The pair-MLP head reuses the pooled gather/one-hot
machinery. The SWDGE gather pipeline (~2.7 ns/row all-in across the 4
queues) is the kernel's roofline; emission keeps it saturated
wall-to-wall between the collectives.
"""

import re

import numpy as np

import concourse.bass as bass
import concourse.bacc as bacc
import concourse.mybir as mybir
import concourse.tile as tile
from concourse import library_config
from concourse.bass_utils import run_bass_kernel_spmd

NC = 8
N_NODES = 100000
F_IN = 256
H = 64
NEG = 0.01

P = 128                    # partitions / tile height
TILES = 98                 # node tiles per core
SHARD = TILES * P          # 12544 nodes per core
NPAD = NC * SHARD          # 100352
SPLITS = (36, 62)          # g AllGather chunk sizes (tiles per core):
                           # small chunk A fires its AG ~30us earlier and
                           # the 62-tile chunk B (31744 pair rows, int16
                           # max) transfers under phase A's tail
HSA = SPLITS[0] * P        # 6144
HSB = SPLITS[1] * P        # 6400
PSPLITS = (56, 42)         # e AllGather chunks (tiles; AG-0 fires after
                           # edge group 6, AG-1 right at edge end; 56
                           # tiles = 28672 pair rows, the int16 max that
                           # still aligns to a group boundary)
NBUCKET = 4                # edge: (g AG chunk) x (src parity)
GROUP = 8                  # node tiles per edge window group
PGROUP = 8                 # pair slot-tiles per window group
PNBUCKET = 4               # pair: (e AG chunk) x (parity)


def _wrap_idx_window(idx):
    """int array [W] (W % 16 == 0) -> [128, W//16] int16 wrapped/replicated."""
    w = idx.reshape(-1, 16).T.astype(np.int16)
    return np.tile(w, (8, 1))


def _node_bucket(n, splits):
    """node id -> (bucket, pair-row in that bucket's table) for an
    AllGather chunking of each core's tiles into `splits` (tile counts)."""
    c = n // SHARD
    off = n % SHARD
    bases = np.concatenate([[0], np.cumsum(splits)]) * P
    a = (np.searchsorted(bases, off, side="right") - 1).astype(np.int64)
    sizes = np.asarray(splits, np.int64) * P
    row = c * sizes[a] + off - bases[a]
    par = n & 1
    return a * 2 + par, row >> 1


def _pooled_sched(core, tl, loc, bucket, prow, ntiles, group_sz,
                  nbucket=NBUCKET):
    """Pooled chunk-stream schedule.

    Items (one per scatter row): destination (core, tile tl, column loc),
    gather source (bucket, prow). Rows are packed per (core, window)
    where window = (tile group, bucket); chunks of 128 rows may span
    tiles -> boundary chunks get one matmul unit per covered tile.
    Unit/chunk structure is shared across cores (max-padded); pad rows
    are trailing -1 indices (SWDGE trims them) with loc=255.
    """
    items = len(core)
    ngroups = (ntiles + group_sz - 1) // group_sz
    grp = tl // group_sz
    tloc = tl - grp * group_sz
    win = grp * nbucket + bucket
    nwin = ngroups * nbucket

    cnt = np.zeros((NC, nwin), np.int64)
    np.add.at(cnt, (core, win), 1)
    K = np.maximum(1, -(-cnt.max(axis=0) // P))        # chunks per window
    woff = np.concatenate([[0], np.cumsum(K)])          # chunk offsets
    totchunks = int(K.sum())
    totidx = totchunks * P

    cnt_t = np.zeros((NC, nwin, group_sz), np.int64)
    np.add.at(cnt_t, (core, win, tloc), 1)
    cum = np.cumsum(cnt_t, axis=2) - cnt_t              # tile start offsets

    units = [set() for _ in range(nwin)]
    for w in range(nwin):
        g = w // nbucket
        tcount = min(group_sz, ntiles - g * group_sz)
        for c in range(NC):
            for tt in range(tcount):
                s, e = cum[c, w, tt], cum[c, w, tt] + cnt_t[c, w, tt]
                if e == s:
                    continue
                for ci in range(s // P, (e - 1) // P + 1):
                    units[w].add((ci, tt))
    units = [sorted(u) for u in units]
    # every tile must appear in >=1 unit per bucket-PAIR (each AG chunk's
    # bucket pair may be consumed as a separate accumulation phase)
    for g in range(ngroups):
        tcount = min(group_sz, ntiles - g * group_sz)
        for half in range(nbucket // 2):
            present = set()
            for b in (2 * half, 2 * half + 1):
                present.update(tt for (_, tt) in units[g * nbucket + b])
            missing = [tt for tt in range(tcount) if tt not in present]
            if missing:
                w0 = g * nbucket + 2 * half
                units[w0].extend((0, tt) for tt in missing)
                units[w0].sort()
    ulen = [len(u) for u in units]
    uoff = np.concatenate([[0], np.cumsum(ulen)]).astype(np.int64)
    totunits = int(uoff[-1])

    kmax = int(K.max())
    lut = np.full((nwin, kmax, group_sz), -1, np.int64)
    for w in range(nwin):
        for i, (ci, tt) in enumerate(units[w]):
            lut[w, ci, tt] = uoff[w] + i

    order = np.lexsort((tl, win, core))
    so_core = core[order]
    so_win = win[order]
    so_tloc = tloc[order]
    so_loc = loc[order]
    so_prow = prow[order]
    key = so_core * nwin + so_win
    starts = np.r_[0, np.flatnonzero(np.diff(key)) + 1]
    run_ids = np.zeros(items, np.int64)
    run_ids[starts[1:]] = 1
    run_ids = np.cumsum(run_ids)
    rank = np.arange(items) - starts[run_ids]
    ci = rank // P
    rr = rank % P
    u = lut[so_win, ci, so_tloc]
    assert (u >= 0).all()

    PAD_TRIM = False
    idx_lin = np.full((NC, totidx), -1 if PAD_TRIM else 0, np.int64)
    loc_arr = np.full((NC, totunits, P), 255, np.int64)
    idx_lin[so_core, (woff[so_win] + ci) * P + rr] = so_prow
    loc_arr[so_core, u, rr] = so_loc

    idx_i16 = np.zeros((NC, P, totidx // 16), np.int16)
    for w in range(nwin):
        lo, hi = woff[w] * P, (woff[w] + K[w]) * P
        for c in range(NC):
            idx_i16[c][:, lo // 16: hi // 16] = _wrap_idx_window(idx_lin[c, lo:hi])
    # per-(row-partition, unit) one-hot column id, built into fp8 one-hots
    # on-device: loc_i16[c][r, u] = column (255 => no contribution)
    loc_i16 = np.ascontiguousarray(loc_arr.transpose(0, 2, 1)).astype(np.int16)

    sched = {
        "ntiles": ntiles,
        "group_sz": group_sz,
        "ngroups": ngroups,
        "nbucket": nbucket,
        "K": K,
        "woff": woff,
        "uoff": uoff,
        "units": units,
        "totchunks": totchunks,
        "totidx": totidx,
        "totunits": totunits,
    }
    return sched, idx_i16, loc_i16


def _emit_flags(sched, phases):
    """flags[(w, i)] = (start, stop) for emitted matmuls: first/last unit
    per tile within each phase (a phase = a list of windows emitted as one
    PSUM accumulation pass)."""
    flags = {}
    nbucket = sched["nbucket"]
    for win_order in phases:
        seen = {}
        for w in win_order:
            g = w // nbucket
            for i, (ci, tt) in enumerate(sched["units"][w]):
                t = g * sched["group_sz"] + tt
                seen.setdefault(t, []).append((w, i))
        for t, lst in seen.items():
            for j, wi in enumerate(lst):
                flags[wi] = (j == 0, j == len(lst) - 1)
    return flags


def _emit_order(ngroups, nbucket, rot):
    """Emission schedule: list of (group, [buckets], [groups to consume
    after]). Plain rotation; group 0's rotation starts at the AG-chunk-A
    buckets so processing begins as soon as AG chunk A lands."""
    return [(g, rot(g), [g]) for g in range(ngroups)]


def _prep(inputs):
    x = np.asarray(inputs["x"], np.float32)
    edge_index = np.asarray(inputs["edge_index"], np.int64)
    index = np.asarray(inputs["index"], np.int64)
    W_conv = np.asarray(inputs["W_conv"], np.float32)
    b_conv = np.asarray(inputs["b_conv"], np.float32)
    W1 = np.asarray(inputs["W1"], np.float32)
    b1 = np.asarray(inputs["b1"], np.float32)
    W2 = np.asarray(inputs["W2"], np.float32)
    b2 = np.asarray(inputs["b2"], np.float32)

    n = x.shape[0]
    src = edge_index[0].astype(np.int64)
    dst = edge_index[1].astype(np.int64)

    # degrees include self-loops (loops handled locally on-device)
    deg = np.bincount(dst, minlength=NPAD).astype(np.float32)
    deg += 1.0
    deg[n:] = 1.0

    # edge buckets: (g AG chunk of src) x parity
    ebucket, eprow = _node_bucket(src, SPLITS)
    esched, eidx, eloc = _pooled_sched(
        core=dst // SHARD, tl=(dst % SHARD) // P, loc=dst % P,
        bucket=ebucket, prow=eprow, ntiles=TILES, group_sz=GROUP)

    # pair stream: per core PB pairs; side slots [xi: 0..PB) [xj: PB..2PB)
    B = index.shape[0]
    PB = B // NC
    assert PB % P == 0
    PCH = PB // P
    pair_global = np.arange(B, dtype=np.int64)
    pcore = pair_global // PB
    plocal = pair_global % PB
    s_core = np.concatenate([pcore, pcore])
    s_slot = np.concatenate([plocal, PB + plocal])
    s_node = np.concatenate([index[:, 0], index[:, 1]]).astype(np.int64)
    pbucket, pprow = _node_bucket(s_node, PSPLITS)
    psched, pidx, ploc = _pooled_sched(
        core=s_core, tl=s_slot // P, loc=s_slot % P,
        bucket=pbucket, prow=pprow, ntiles=2 * PCH, group_sz=PGROUP,
        nbucket=PNBUCKET)

    xpad = np.zeros((NPAD, F_IN), np.float32)
    xpad[:n] = x
    xT = xpad.T.astype(np.float16)
    xT_shards = [
        np.ascontiguousarray(
            xT[:, c * SHARD:(c + 1) * SHARD].reshape(2, P, SHARD).transpose(1, 0, 2)
        ) for c in range(NC)
    ]
    deg_sb = [
        np.ascontiguousarray(
            deg[c * SHARD:(c + 1) * SHARD].reshape(TILES, P).T
        ) for c in range(NC)
    ]

    iota = np.tile(np.arange(P, dtype=np.int16), (P, 1))
    consts = {
        "wc": np.ascontiguousarray(
            W_conv.reshape(2, P, H).transpose(1, 0, 2)).astype(np.float16),
        "bconvb": np.broadcast_to(b_conv, (P, H)).astype(np.float32).copy(),
        "ident": np.eye(P, dtype=np.float16),
        "iota": iota,
        "w1": W1.astype(np.float16),
        "b1": b1.reshape(16, 1).astype(np.float32),
        "w2": W2.astype(np.float32),
        "b2t": b2.reshape(1, 1).astype(np.float32),
    }
    sched = {"edge": esched, "pair": psched, "PCH": PCH}
    in_maps = []
    for c in range(NC):
        m = {
            "xt": xT_shards[c],
            "deg": deg_sb[c],
            "eidx": eidx[c],
            "pidx": pidx[c],
            "eloc": eloc[c],
            "ploc": ploc[c],
        }
        m.update(consts)
        in_maps.append(m)
    return in_maps, sched


def _build(sched):
    dt = mybir.dt
    esched = sched["edge"]
    psched = sched["pair"]
    PCH = sched["PCH"]

    nc = bacc.Bacc("TRN2", target_bir_lowering=False, debug=False,
                   enable_asserts=False, num_devices=NC, num_swdge_queues=4)

    xt_in = nc.dram_tensor("xt", [P, 2, SHARD], dt.float16, kind="ExternalInput")
    deg_in = nc.dram_tensor("deg", [P, TILES], dt.float32, kind="ExternalInput")
    eidx_in = nc.dram_tensor("eidx", [P, esched["totidx"] // 16], dt.int16,
                             kind="ExternalInput")
    pidx_in = nc.dram_tensor("pidx", [P, psched["totidx"] // 16], dt.int16,
                             kind="ExternalInput")
    eloc_in = nc.dram_tensor("eloc", [P, esched["totunits"]], dt.int16,
                             kind="ExternalInput")
    ploc_in = nc.dram_tensor("ploc", [P, psched["totunits"]], dt.int16,
                             kind="ExternalInput")
    wc_in = nc.dram_tensor("wc", [P, 2, H], dt.float16, kind="ExternalInput")
    bconvb_in = nc.dram_tensor("bconvb", [P, H], dt.float32, kind="ExternalInput")
    ident_in = nc.dram_tensor("ident", [P, P], dt.float16, kind="ExternalInput")
    iota_in = nc.dram_tensor("iota", [P, P], dt.int16, kind="ExternalInput")
    w1_in = nc.dram_tensor("w1", [P, 16], dt.float16, kind="ExternalInput")
    b1_in = nc.dram_tensor("b1", [16, 1], dt.float32, kind="ExternalInput")
    w2_in = nc.dram_tensor("w2", [16, 1], dt.float32, kind="ExternalInput")
    b2_in = nc.dram_tensor("b2t", [1, 1], dt.float32, kind="ExternalInput")
    outp = nc.dram_tensor("out", [PCH * P, 1], dt.float32, kind="ExternalOutput")

    g_shardA = nc.dram_tensor("g_shardA", [HSA, H], dt.float16)
    g_shardB = nc.dram_tensor("g_shardB", [HSB, H], dt.float16)
    g_fullA = nc.dram_tensor("g_fullA", [NC * HSA, H], dt.float16,
                             addr_space="Shared")
    g_fullB = nc.dram_tensor("g_fullB", [NC * HSB, H], dt.float16,
                             addr_space="Shared")
    e_shards = [
        nc.dram_tensor(f"e_shard{i}", [PSPLITS[i] * P, H], dt.float16)
        for i in range(2)
    ]
    e_fulls = [
        nc.dram_tensor(f"e_full{i}", [NC * PSPLITS[i] * P, H], dt.float16,
                       addr_space="Shared")
        for i in range(2)
    ]

    g_tabs = [
        g_fullA[:, :].rearrange("(r two) f -> r (two f)", two=2),
        g_fullB[:, :].rearrange("(r two) f -> r (two f)", two=2),
    ]
    e_tabs = [
        ef[:, :].rearrange("(r two) f -> r (two f)", two=2) for ef in e_fulls
    ]

    def erot(g):
        r = (g + g // 4) % NBUCKET
        return [(r + j) % NBUCKET for j in range(NBUCKET)]

    def prot(pg):
        return [(pg + j) % PNBUCKET for j in range(PNBUCKET)]

    ngr = esched["ngroups"]
    e_entries = [(g, erot(g), [("eR", g)]) for g in range(ngr)]
    ephase = [g * NBUCKET + b for (g, bs, _) in e_entries for b in bs]
    eflags = _emit_flags(esched, [ephase])
    # pair: one PSUM accumulation phase per e-AG chunk; chunk A is copied
    # into xs, later chunks are added (so chunk-k gathers run while the
    # next chunk's AllGather is still in flight)
    p_entries = []
    for ph in range(2):
        for pg in range(psched["ngroups"]):
            bs = [ph * 2 + (pg & 1), ph * 2 + 1 - (pg & 1)]
            p_entries.append((pg, bs, [(pg, ph)]))
    pphases = [
        [pg * PNBUCKET + b for (pg, bs, _) in p_entries for b in bs
         if b // 2 == ph]
        for ph in range(2)
    ]
    pflags = _emit_flags(psched, pphases)

    with tile.TileContext(nc) as tc:
        nc.gpsimd.load_library(library_config.mlp)

        with (
            tc.tile_pool(name="const", bufs=1) as cpool,
            tc.tile_pool(name="dinvp", bufs=1) as dpool,
            tc.tile_pool(name="gres", bufs=1) as gpool,
        ):
            wc_sb = cpool.tile([P, 2, H], dt.float16)
            nc.sync.dma_start(wc_sb[:], wc_in[:, :, :])
            bconvb = cpool.tile([P, H], dt.float32)
            nc.sync.dma_start(bconvb[:], bconvb_in[:, :])
            ident = cpool.tile([P, P], dt.float16)
            nc.sync.dma_start(ident[:], ident_in[:, :])
            iota_sb = cpool.tile([P, P], dt.int16)
            nc.sync.dma_start(iota_sb[:], iota_in[:, :])
            w1_sb = cpool.tile([P, 16], dt.float16)
            nc.sync.dma_start(w1_sb[:], w1_in[:, :])
            b1_sb = cpool.tile([16, 1], dt.float32)
            nc.sync.dma_start(b1_sb[:], b1_in[:, :])
            w2_sb = cpool.tile([16, 1], dt.float32)
            nc.sync.dma_start(w2_sb[:], w2_in[:, :])
            b2_sb = cpool.tile([1, 1], dt.float32)
            nc.sync.dma_start(b2_sb[:], b2_in[:, :])
            deg_sb = dpool.tile([P, TILES], dt.float32)
            nc.sync.dma_start(deg_sb[:], deg_in[:, :])
            sq = dpool.tile([P, TILES], dt.float32)
            nc.scalar.activation(sq[:], deg_sb[:], mybir.ActivationFunctionType.Sqrt)
            dinv = dpool.tile([P, TILES], dt.float32)
            nc.vector.reciprocal(dinv[:], sq[:])

            g_sb = gpool.tile([P, TILES, H], dt.float16)
            g2_sb = gpool.tile([P, TILES, H], dt.float16)
            e_sb = g_sb  # g is dead once G2 is built; reuse its SBUF for e

            # ---------------- phase A: g = (x @ W) * dinv ----------------
            with (
                tc.tile_pool(name="xtp", bufs=2) as xtp,
                tc.tile_pool(name="hps", bufs=4, space="PSUM") as hps,
            ):
                blocks = [(0, 18), (18, 36), (36, 57),
                          (57, 78), (78, TILES)]
                for bi, (t0, t1) in enumerate(blocks):
                    xt_sb = xtp.tile([P, 2, (t1 - t0) * P], dt.float16, tag="xt")
                    eng = nc.sync if bi % 2 == 0 else nc.scalar
                    eng.dma_start(xt_sb[:], xt_in[:, :, t0 * P: t1 * P])
                    for t in range(t0, t1):
                        h_ps = hps.tile([P, H], dt.float32)
                        for k in range(2):
                            nc.tensor.matmul(
                                h_ps[:],
                                lhsT=xt_sb[:, k, (t - t0) * P:(t - t0 + 1) * P],
                                rhs=wc_sb[:, k, :],
                                start=(k == 0), stop=(k == 1),
                            )
                        nc.vector.tensor_scalar(
                            g_sb[:, t, :], h_ps[:], dinv[:, t:t + 1], None,
                            mybir.AluOpType.mult,
                        )
                    # write this block's g slice out immediately so the
                    # chunk AllGather can trigger as soon as its tiles land
                    shard, base = ((g_shardA, 0) if t1 <= SPLITS[0]
                                   else (g_shardB, SPLITS[0]))
                    weng = nc.scalar if bi % 2 == 0 else nc.sync
                    weng.dma_start(
                        shard[(t0 - base) * P:(t1 - base) * P, :].rearrange(
                            "(t p) f -> p t f", p=P),
                        g_sb[:, t0:t1, :],
                    )

                    if t1 == SPLITS[0]:
                        nc.gpsimd.collective_compute(
                            "AllGather", mybir.AluOpType.bypass,
                            replica_groups=[list(range(NC))],
                            ins=[g_shardA[:, :].opt()],
                            outs=[g_fullA[:, :].opt()],
                        )
                nc.gpsimd.collective_compute(
                    "AllGather", mybir.AluOpType.bypass,
                    replica_groups=[list(range(NC))],
                    ins=[g_shardB[:, :].opt()],
                    outs=[g_fullB[:, :].opt()],
                )
                # index/loc tables load behind phase A's x blocks; the first
                # gather needs them only once the g AllGather lands
                eidx_sb = cpool.tile([P, esched["totidx"] // 16], dt.int16)
                nc.scalar.dma_start(eidx_sb[:], eidx_in[:, :])
                pidx_sb = cpool.tile([P, psched["totidx"] // 16], dt.int16)
                nc.scalar.dma_start(pidx_sb[:], pidx_in[:, :])
                eloc_sb = cpool.tile([P, esched["totunits"]], dt.int16)
                nc.scalar.dma_start(eloc_sb[:], eloc_in[:, :])
                ploc_sb = cpool.tile([P, psched["totunits"]], dt.int16)
                nc.scalar.dma_start(ploc_sb[:], ploc_in[:, :])
                # G2 = g*dinv + b_conv (consume becomes acc*dinv + G2);
                # computed under the AllGathers, off the critical path
                for t in range(TILES):
                    nc.vector.tensor_scalar(
                        g2_sb[:, t, :], g_sb[:, t, :], dinv[:, t:t + 1],
                        None, mybir.AluOpType.mult,
                    )
                    nc.vector.tensor_tensor(
                        g2_sb[:, t, :], g2_sb[:, t, :], bconvb[:],
                        mybir.AluOpType.add,
                    )

            # ---------------- phase C: aggregate per dst tile ----------------
            EMSG_BUFS = 2
            with (
                tc.tile_pool(name="emsg", bufs=EMSG_BUFS) as msgp,
                tc.tile_pool(name="eoh", bufs=2) as ohp,
                tc.tile_pool(name="epost", bufs=4) as postp,
                tc.tile_pool(name="pmsg", bufs=2) as pmsgp,
                tc.tile_pool(name="poh", bufs=2) as pohp,
                tc.tile_pool(name="pxs", bufs=1) as pxsp,
            ):
                xs_sb = pxsp.tile([P, psched["ntiles"], H], dt.float16)
                ngroups_e = esched["ngroups"]
                ngroups_p = psched["ngroups"]

                def emit_gather(sched_, idx_sb, tabs, mp, w, prefix):
                    nb = sched_["nbucket"]
                    b = w % nb
                    K = int(sched_["K"][w])
                    ioff = int(sched_["woff"][w]) * P
                    msg = mp.tile([P, K, P], dt.float16, tag=f"{prefix}m{b}",
                                  name=f"{prefix}msg_w{w}")
                    # split large windows into two gathers: descriptors only
                    # trigger at gen end, so half 1 drains under half 2's gen
                    halves = [(0, K // 2), (K // 2, K)] if K >= 8 else [(0, K)]
                    for (k0, k1) in halves:
                        if k0 == k1:
                            continue
                        hn = (k1 - k0) * P
                        hoff = ioff + k0 * P
                        nc.gpsimd.dma_gather(
                            msg[:, k0:k1, :], tabs[b // 2],
                            idx_sb[:, hoff // 16:(hoff + hn) // 16],
                            hn, hn, P, single_packet=False, queue_num=0)
                    return msg

                def emit_ohgen(sched_, loc_sb, op_, w, prefix):
                    # one-hot built on-device: iota is_equal the per-unit
                    # column table (255 => all-zero row, no contribution)
                    b = w % sched_["nbucket"]
                    uoff = int(sched_["uoff"][w])
                    nu = len(sched_["units"][w])
                    if nu == 0:
                        return None
                    oh = op_.tile([P, nu, P], dt.float8e4,
                                  tag=f"{prefix}o{b}" if prefix == "e"
                                  else f"po{b & 1}",
                                  name=f"{prefix}oh_w{w}")
                    nc.vector.tensor_tensor(
                        oh[:],
                        iota_sb[:].unsqueeze(1).broadcast_to([P, nu, P]),
                        loc_sb[:, uoff:uoff + nu].unsqueeze(2).broadcast_to(
                            [P, nu, P]),
                        mybir.AluOpType.is_equal,
                    )
                    return oh

                def emit_mms(sched_, flags, ap, acc_tiles, w, prefix,
                             msg, oh):
                    nb = sched_["nbucket"]
                    g = w // nb
                    b = w % nb
                    par = b & 1
                    for i, (ci, tt) in enumerate(sched_["units"][w]):
                        t = g * sched_["group_sz"] + tt
                        if t not in acc_tiles:
                            acc_tiles[t] = ap.tile(
                                [P, H], dt.float32,
                                tag=f"a{t % sched_['group_sz']}",
                                name=f"{prefix}acc_t{t}_w{w}")
                        st, sp = flags[(w, i)]
                        nc.tensor.matmul(
                            acc_tiles[t][:],
                            lhsT=oh[:, i, :],
                            rhs=msg[:, ci, par * H:(par + 1) * H],
                            start=st, stop=sp,
                        )

                def consume_edge(t, a):
                    # e = leaky(acc*dinv + G2); all-DVE to avoid cross-engine
                    # sem round-trips inside the serialized vector FIFO
                    s1 = postp.tile([P, H], dt.float32, tag="e1", name=f"e1_{t}")
                    nc.vector.tensor_scalar(
                        s1[:], a[:], dinv[:, t:t + 1], None, mybir.AluOpType.mult)
                    nc.vector.tensor_tensor(
                        s1[:], s1[:], g2_sb[:, t, :], mybir.AluOpType.add)
                    m = postp.tile([P, H], dt.float32, tag="m", name=f"m_{t}")
                    nc.vector.tensor_scalar(
                        m[:], s1[:], NEG, None, mybir.AluOpType.mult)
                    nc.vector.tensor_tensor(
                        e_sb[:, t, :], s1[:], m[:], mybir.AluOpType.max)

                with tc.tile_pool(name="eacc", bufs=1, space="PSUM") as accp:
                    # unified emission: pair chunk-k entries are interleaved
                    # into the edge stream as soon as e-AG chunk k has fired,
                    # so the SWDGE sequencer (the kernel's bottleneck) never
                    # idles behind a collective. The last e-AG (chunk 2) is
                    # triggered only after pair chunk-1 gathers are queued.
                    ekind = {
                        "sched": esched, "idx": eidx_sb, "loc": eloc_sb,
                        "tabs": g_tabs, "flags": eflags, "msgp": msgp,
                        "ohp": ohp, "nb": NBUCKET, "prefix": "e",
                    }
                    pkind = {
                        "sched": psched, "idx": pidx_sb, "loc": ploc_sb,
                        "tabs": e_tabs, "flags": pflags, "msgp": pmsgp,
                        "ohp": pohp, "nb": PNBUCKET, "prefix": "p",
                    }
                    p_by_phase = [[], []]
                    for ent in p_entries:
                        p_by_phase[ent[1][0] // 2].append(
                            (pkind, ent[0], ent[1],
                             [("p",) + c for c in ent[2]]))
                    e_seq = [(ekind,) + ent for ent in e_entries]
                    seq = (e_seq + [None] + p_by_phase[0] + p_by_phase[1])

                    def emit_e_consume(cg):
                        acc_c = group_acc.pop(("e", cg, 1))
                        t0 = cg * GROUP
                        t1 = min(t0 + GROUP, TILES)
                        for t in range(t0, t1):
                            consume_edge(t, acc_c.pop(t)[:])
                        # stream this group's e rows out right away; e-AG
                        # chunks 0/1 fire as soon as their last group lands
                        pb = np.concatenate([[0], np.cumsum(PSPLITS)])
                        ci_ = int(np.searchsorted(pb, t0, side="right")) - 1
                        base = int(pb[ci_])
                        nc.sync.dma_start(
                            e_shards[ci_][(t0 - base) * P:(t1 - base) * P, :]
                            .rearrange("(t p) f -> p t f", p=P),
                            e_sb[:, t0:t1, :],
                        )
                        if t1 == base + PSPLITS[ci_] and ci_ < 1:
                            nc.gpsimd.collective_compute(
                                "AllGather", mybir.AluOpType.bypass,
                                replica_groups=[list(range(NC))],
                                ins=[e_shards[ci_][:, :].opt()],
                                outs=[e_fulls[ci_][:, :].opt()],
                            )

                    def gen_ohs(ent):
                        k = ent[0]
                        g, bs = ent[1], ent[2]
                        for b in bs:
                            ohs[(k["prefix"], g * k["nb"] + b)] = emit_ohgen(
                                k["sched"], k["loc"], k["ohp"],
                                g * k["nb"] + b, k["prefix"])

                    ohs = {}
                    group_acc = {}
                    real = [s for s in seq if s is not None]
                    gen_ohs(real[0])
                    ri = 0
                    for ent in seq:
                        if ent is None:
                            nc.gpsimd.collective_compute(
                                "AllGather", mybir.AluOpType.bypass,
                                replica_groups=[list(range(NC))],
                                ins=[e_shards[1][:, :].opt()],
                                outs=[e_fulls[1][:, :].opt()],
                            )
                            continue
                        k, g, bs, consume = ent
                        pref, nb = k["prefix"], k["nb"]
                        if pref == "e":
                            acc_tiles = group_acc.setdefault(
                                ("e", g, 1), {})
                        else:
                            acc_tiles = group_acc.setdefault(
                                ("p", g, bs[0] // 2), {})
                        msgs = {}
                        for b in bs:
                            msgs[b] = emit_gather(k["sched"], k["idx"],
                                                  k["tabs"], k["msgp"],
                                                  g * nb + b, pref)
                        ri += 1
                        if ri < len(real):
                            gen_ohs(real[ri])
                        for b in bs:
                            oh = ohs.pop((pref, g * nb + b))
                            if oh is not None:
                                emit_mms(k["sched"], k["flags"], accp,
                                         acc_tiles, g * nb + b, pref,
                                         msgs[b], oh)
                        for c in consume:
                            if c[0] == "eR":
                                emit_e_consume(c[1])
                            else:
                                _, cg, ph = c
                                acc_c = group_acc.pop(("p", cg, ph))
                                for t in range(cg * PGROUP,
                                               (cg + 1) * PGROUP):
                                    if ph == 0:
                                        nc.vector.tensor_copy(
                                            xs_sb[:, t, :], acc_c.pop(t)[:])
                                    else:
                                        nc.vector.tensor_tensor(
                                            xs_sb[:, t, :], xs_sb[:, t, :],
                                            acc_c.pop(t)[:],
                                            mybir.AluOpType.add)

                # ---------------- phase D: pair MLP head ----------------
                with (
                    tc.tile_pool(name="ptps", bufs=4, space="PSUM") as ptps,
                    tc.tile_pool(name="pzps", bufs=1, space="PSUM") as pzps,
                    tc.tile_pool(name="pops", bufs=1, space="PSUM") as pops,
                    tc.tile_pool(name="psb", bufs=2) as psbp,
                ):
                    for k in range(PCH):
                        xt_ps = ptps.tile([P, P], dt.float16)
                        nc.tensor.transpose(xt_ps[0:H, :], xs_sb[:, k, :], ident[:])
                        nc.tensor.transpose(xt_ps[H:P, :], xs_sb[:, PCH + k, :],
                                            ident[:])
                        xijt = psbp.tile([P, P], dt.float16, tag="xijt")
                        nc.vector.tensor_copy(xijt[:], xt_ps[:])
                        z_ps = pzps.tile([16, P], dt.float32)
                        nc.tensor.matmul(z_ps[:], lhsT=w1_sb[:], rhs=xijt[:],
                                         start=True, stop=True)
                        zb = psbp.tile([16, P], dt.float32, tag="zb")
                        nc.vector.tensor_scalar(
                            zb[:], z_ps[:], b1_sb[:, 0:1], None, mybir.AluOpType.add)
                        m2 = psbp.tile([16, P], dt.float32, tag="m2")
                        nc.vector.tensor_scalar(
                            m2[:], zb[:], NEG, None, mybir.AluOpType.mult)
                        z2 = psbp.tile([16, P], dt.float32, tag="z2")
                        nc.vector.tensor_tensor(z2[:], zb[:], m2[:],
                                                mybir.AluOpType.max)
                        o_ps = pops.tile([1, P], dt.float32)
                        nc.tensor.matmul(o_ps[:], lhsT=w2_sb[:], rhs=z2[:],
                                         start=True, stop=True)
                        osb = psbp.tile([1, P], dt.float32, tag="osb")
                        nc.scalar.activation(
                            osb[:], o_ps[:], mybir.ActivationFunctionType.Sigmoid,
                            bias=b2_sb[:, 0:1], scale=1.0)
                        nc.sync.dma_start(
                            outp[k * P:(k + 1) * P, :].rearrange("r one -> one r"),
                            osb[0:1, :])

    # align each gather's SWDGE queue with its Tile-assigned DMA lane so
    # semaphore<->queue locking stays consistent (4-way parallel desc gen)
    for blk in nc.m.functions[0].blocks:
        for inst in blk.instructions:
            if isinstance(inst, mybir.InstDMAGatherAnt):
                si = inst.sync_info
                for u in (si.on_update if si else []):
                    mm = re.match(r"DMASW(\d+)_", u.ant_name or "")
                    if mm:
                        inst.queue_num = int(mm.group(1)) % 4
                        break

    nc.compile()
    return nc


def kernel(**inputs) -> np.ndarray:
    in_maps, sched = _prep(inputs)
    nc = _build(sched)
    res = run_bass_kernel_spmd(nc, in_maps, list(range(NC)))
    out = np.concatenate([res.results[c]["out"] for c in range(NC)], axis=0)
    return out.astype(np.float32)


# revision 46
# speedup vs baseline: 1.0712x; 1.0712x over previous
"""GCN message-passing kernel for 8 Trainium2 NeuronCores (Bass/Tile).

Computes (matching the jax reference):
    h = x @ W_conv                      [N, H]
    node_embed = leaky_relu(D^-1/2 (A+I) D^-1/2 h + b_conv)
    out = sigmoid(leaky(cat(e[i], e[j]) @ W1 + b1) @ W2 + b2)

Distribution: nodes dst-sharded over the 8 cores. The scaled features
g = dinv * h are AllGathered in TWO chunks (tiles 0-47 / 48-97) so the
second chunk's collective overlaps the first chunk's edge processing;
gather buckets are keyed (AG chunk x src parity) so chunk-A windows
start as soon as AG-A lands. Per-edge source rows are fetched with bulk
SWDGE dma_gather (pair-packed fp16 rows; int16 indices stay in range
because each chunk table has <=25600 pair rows; 4 parallel SWDGE
queues; each window split into two half-gathers so descriptor drain
overlaps generation). Edges are packed into pooled per-(group,bucket)
chunk streams and scatter-added on the TensorEngine via fp8 one-hot
matmuls GENERATED ON-DEVICE (one DVE is_equal against an iota tile per
window) instead of streamed from DRAM. Self-loop contributions are
added locally from the resident g tiles (an all-DVE leaky chain). The
node embeddings are AllGathered with the same 2-chunk overlap: chunk A
fires mid-edge-phase, chunk B right at edge end, and the pair stream is
split into two PSUM accumulation phases (chunk-A windows copied into
xs, chunk-B windows added) so pair chunk-A gathers run underneath the
final e AllGather. The pair-MLP head reuses the pooled gather/one-hot
machinery. The SWDGE gather pipeline (~2.7 ns/row all-in across the 4
queues) is the kernel's roofline; emission keeps it saturated
wall-to-wall between the collectives.
"""

import re

import numpy as np

import concourse.bass as bass
import concourse.bacc as bacc
import concourse.mybir as mybir
import concourse.tile as tile
from concourse import library_config
from concourse.bass_utils import run_bass_kernel_spmd

NC = 8
N_NODES = 100000
F_IN = 256
H = 64
NEG = 0.01

P = 128                    # partitions / tile height
TILES = 98                 # node tiles per core
SHARD = TILES * P          # 12544 nodes per core
NPAD = NC * SHARD          # 100352
SPLITS = (48, 50)          # g AllGather chunk sizes (tiles per core)
HSA = SPLITS[0] * P        # 6144
HSB = SPLITS[1] * P        # 6400
PSPLITS = (56, 42)         # e AllGather chunks (tiles; AG-0 fires after
                           # edge group 6, AG-1 right at edge end; 56
                           # tiles = 28672 pair rows, the int16 max that
                           # still aligns to a group boundary)
NBUCKET = 4                # edge: (g AG chunk) x (src parity)
GROUP = 8                  # node tiles per edge window group
PGROUP = 8                 # pair slot-tiles per window group
PNBUCKET = 4               # pair: (e AG chunk) x (parity)


def _wrap_idx_window(idx):
    """int array [W] (W % 16 == 0) -> [128, W//16] int16 wrapped/replicated."""
    w = idx.reshape(-1, 16).T.astype(np.int16)
    return np.tile(w, (8, 1))


def _node_bucket(n, splits):
    """node id -> (bucket, pair-row in that bucket's table) for an
    AllGather chunking of each core's tiles into `splits` (tile counts)."""
    c = n // SHARD
    off = n % SHARD
    bases = np.concatenate([[0], np.cumsum(splits)]) * P
    a = (np.searchsorted(bases, off, side="right") - 1).astype(np.int64)
    sizes = np.asarray(splits, np.int64) * P
    row = c * sizes[a] + off - bases[a]
    par = n & 1
    return a * 2 + par, row >> 1


def _pooled_sched(core, tl, loc, bucket, prow, ntiles, group_sz,
                  nbucket=NBUCKET):
    """Pooled chunk-stream schedule.

    Items (one per scatter row): destination (core, tile tl, column loc),
    gather source (bucket, prow). Rows are packed per (core, window)
    where window = (tile group, bucket); chunks of 128 rows may span
    tiles -> boundary chunks get one matmul unit per covered tile.
    Unit/chunk structure is shared across cores (max-padded); pad rows
    are trailing -1 indices (SWDGE trims them) with loc=255.
    """
    items = len(core)
    ngroups = (ntiles + group_sz - 1) // group_sz
    grp = tl // group_sz
    tloc = tl - grp * group_sz
    win = grp * nbucket + bucket
    nwin = ngroups * nbucket

    cnt = np.zeros((NC, nwin), np.int64)
    np.add.at(cnt, (core, win), 1)
    K = np.maximum(1, -(-cnt.max(axis=0) // P))        # chunks per window
    woff = np.concatenate([[0], np.cumsum(K)])          # chunk offsets
    totchunks = int(K.sum())
    totidx = totchunks * P

    cnt_t = np.zeros((NC, nwin, group_sz), np.int64)
    np.add.at(cnt_t, (core, win, tloc), 1)
    cum = np.cumsum(cnt_t, axis=2) - cnt_t              # tile start offsets

    units = [set() for _ in range(nwin)]
    for w in range(nwin):
        g = w // nbucket
        tcount = min(group_sz, ntiles - g * group_sz)
        for c in range(NC):
            for tt in range(tcount):
                s, e = cum[c, w, tt], cum[c, w, tt] + cnt_t[c, w, tt]
                if e == s:
                    continue
                for ci in range(s // P, (e - 1) // P + 1):
                    units[w].add((ci, tt))
    units = [sorted(u) for u in units]
    # every tile must appear in >=1 unit per bucket-PAIR (each AG chunk's
    # bucket pair may be consumed as a separate accumulation phase)
    for g in range(ngroups):
        tcount = min(group_sz, ntiles - g * group_sz)
        for half in range(nbucket // 2):
            present = set()
            for b in (2 * half, 2 * half + 1):
                present.update(tt for (_, tt) in units[g * nbucket + b])
            missing = [tt for tt in range(tcount) if tt not in present]
            if missing:
                w0 = g * nbucket + 2 * half
                units[w0].extend((0, tt) for tt in missing)
                units[w0].sort()
    ulen = [len(u) for u in units]
    uoff = np.concatenate([[0], np.cumsum(ulen)]).astype(np.int64)
    totunits = int(uoff[-1])

    kmax = int(K.max())
    lut = np.full((nwin, kmax, group_sz), -1, np.int64)
    for w in range(nwin):
        for i, (ci, tt) in enumerate(units[w]):
            lut[w, ci, tt] = uoff[w] + i

    order = np.lexsort((tl, win, core))
    so_core = core[order]
    so_win = win[order]
    so_tloc = tloc[order]
    so_loc = loc[order]
    so_prow = prow[order]
    key = so_core * nwin + so_win
    starts = np.r_[0, np.flatnonzero(np.diff(key)) + 1]
    run_ids = np.zeros(items, np.int64)
    run_ids[starts[1:]] = 1
    run_ids = np.cumsum(run_ids)
    rank = np.arange(items) - starts[run_ids]
    ci = rank // P
    rr = rank % P
    u = lut[so_win, ci, so_tloc]
    assert (u >= 0).all()

    PAD_TRIM = False
    idx_lin = np.full((NC, totidx), -1 if PAD_TRIM else 0, np.int64)
    loc_arr = np.full((NC, totunits, P), 255, np.int64)
    idx_lin[so_core, (woff[so_win] + ci) * P + rr] = so_prow
    loc_arr[so_core, u, rr] = so_loc

    idx_i16 = np.zeros((NC, P, totidx // 16), np.int16)
    for w in range(nwin):
        lo, hi = woff[w] * P, (woff[w] + K[w]) * P
        for c in range(NC):
            idx_i16[c][:, lo // 16: hi // 16] = _wrap_idx_window(idx_lin[c, lo:hi])
    # per-(row-partition, unit) one-hot column id, built into fp8 one-hots
    # on-device: loc_i16[c][r, u] = column (255 => no contribution)
    loc_i16 = np.ascontiguousarray(loc_arr.transpose(0, 2, 1)).astype(np.int16)

    sched = {
        "ntiles": ntiles,
        "group_sz": group_sz,
        "ngroups": ngroups,
        "nbucket": nbucket,
        "K": K,
        "woff": woff,
        "uoff": uoff,
        "units": units,
        "totchunks": totchunks,
        "totidx": totidx,
        "totunits": totunits,
    }
    return sched, idx_i16, loc_i16


def _emit_flags(sched, phases):
    """flags[(w, i)] = (start, stop) for emitted matmuls: first/last unit
    per tile within each phase (a phase = a list of windows emitted as one
    PSUM accumulation pass)."""
    flags = {}
    nbucket = sched["nbucket"]
    for win_order in phases:
        seen = {}
        for w in win_order:
            g = w // nbucket
            for i, (ci, tt) in enumerate(sched["units"][w]):
                t = g * sched["group_sz"] + tt
                seen.setdefault(t, []).append((w, i))
        for t, lst in seen.items():
            for j, wi in enumerate(lst):
                flags[wi] = (j == 0, j == len(lst) - 1)
    return flags


def _emit_order(ngroups, nbucket, rot):
    """Emission schedule: list of (group, [buckets], [groups to consume
    after]). Plain rotation; group 0's rotation starts at the AG-chunk-A
    buckets so processing begins as soon as AG chunk A lands."""
    return [(g, rot(g), [g]) for g in range(ngroups)]


def _prep(inputs):
    x = np.asarray(inputs["x"], np.float32)
    edge_index = np.asarray(inputs["edge_index"], np.int64)
    index = np.asarray(inputs["index"], np.int64)
    W_conv = np.asarray(inputs["W_conv"], np.float32)
    b_conv = np.asarray(inputs["b_conv"], np.float32)
    W1 = np.asarray(inputs["W1"], np.float32)
    b1 = np.asarray(inputs["b1"], np.float32)
    W2 = np.asarray(inputs["W2"], np.float32)
    b2 = np.asarray(inputs["b2"], np.float32)

    n = x.shape[0]
    src = edge_index[0].astype(np.int64)
    dst = edge_index[1].astype(np.int64)

    # degrees include self-loops (loops handled locally on-device)
    deg = np.bincount(dst, minlength=NPAD).astype(np.float32)
    deg += 1.0
    deg[n:] = 1.0

    # edge buckets: (g AG chunk of src) x parity
    ebucket, eprow = _node_bucket(src, SPLITS)
    esched, eidx, eloc = _pooled_sched(
        core=dst // SHARD, tl=(dst % SHARD) // P, loc=dst % P,
        bucket=ebucket, prow=eprow, ntiles=TILES, group_sz=GROUP)

    # pair stream: per core PB pairs; side slots [xi: 0..PB) [xj: PB..2PB)
    B = index.shape[0]
    PB = B // NC
    assert PB % P == 0
    PCH = PB // P
    pair_global = np.arange(B, dtype=np.int64)
    pcore = pair_global // PB
    plocal = pair_global % PB
    s_core = np.concatenate([pcore, pcore])
    s_slot = np.concatenate([plocal, PB + plocal])
    s_node = np.concatenate([index[:, 0], index[:, 1]]).astype(np.int64)
    pbucket, pprow = _node_bucket(s_node, PSPLITS)
    psched, pidx, ploc = _pooled_sched(
        core=s_core, tl=s_slot // P, loc=s_slot % P,
        bucket=pbucket, prow=pprow, ntiles=2 * PCH, group_sz=PGROUP,
        nbucket=PNBUCKET)

    xpad = np.zeros((NPAD, F_IN), np.float32)
    xpad[:n] = x
    xT = xpad.T.astype(np.float16)
    xT_shards = [
        np.ascontiguousarray(
            xT[:, c * SHARD:(c + 1) * SHARD].reshape(2, P, SHARD).transpose(1, 0, 2)
        ) for c in range(NC)
    ]
    deg_sb = [
        np.ascontiguousarray(
            deg[c * SHARD:(c + 1) * SHARD].reshape(TILES, P).T
        ) for c in range(NC)
    ]

    iota = np.tile(np.arange(P, dtype=np.int16), (P, 1))
    consts = {
        "wc": np.ascontiguousarray(
            W_conv.reshape(2, P, H).transpose(1, 0, 2)).astype(np.float16),
        "bconvb": np.broadcast_to(b_conv, (P, H)).astype(np.float32).copy(),
        "ident": np.eye(P, dtype=np.float16),
        "iota": iota,
        "w1": W1.astype(np.float16),
        "b1": b1.reshape(16, 1).astype(np.float32),
        "w2": W2.astype(np.float32),
        "b2t": b2.reshape(1, 1).astype(np.float32),
    }
    sched = {"edge": esched, "pair": psched, "PCH": PCH}
    in_maps = []
    for c in range(NC):
        m = {
            "xt": xT_shards[c],
            "deg": deg_sb[c],
            "eidx": eidx[c],
            "pidx": pidx[c],
            "eloc": eloc[c],
            "ploc": ploc[c],
        }
        m.update(consts)
        in_maps.append(m)
    return in_maps, sched


def _build(sched):
    dt = mybir.dt
    esched = sched["edge"]
    psched = sched["pair"]
    PCH = sched["PCH"]

    nc = bacc.Bacc("TRN2", target_bir_lowering=False, debug=False,
                   enable_asserts=False, num_devices=NC, num_swdge_queues=4)

    xt_in = nc.dram_tensor("xt", [P, 2, SHARD], dt.float16, kind="ExternalInput")
    deg_in = nc.dram_tensor("deg", [P, TILES], dt.float32, kind="ExternalInput")
    eidx_in = nc.dram_tensor("eidx", [P, esched["totidx"] // 16], dt.int16,
                             kind="ExternalInput")
    pidx_in = nc.dram_tensor("pidx", [P, psched["totidx"] // 16], dt.int16,
                             kind="ExternalInput")
    eloc_in = nc.dram_tensor("eloc", [P, esched["totunits"]], dt.int16,
                             kind="ExternalInput")
    ploc_in = nc.dram_tensor("ploc", [P, psched["totunits"]], dt.int16,
                             kind="ExternalInput")
    wc_in = nc.dram_tensor("wc", [P, 2, H], dt.float16, kind="ExternalInput")
    bconvb_in = nc.dram_tensor("bconvb", [P, H], dt.float32, kind="ExternalInput")
    ident_in = nc.dram_tensor("ident", [P, P], dt.float16, kind="ExternalInput")
    iota_in = nc.dram_tensor("iota", [P, P], dt.int16, kind="ExternalInput")
    w1_in = nc.dram_tensor("w1", [P, 16], dt.float16, kind="ExternalInput")
    b1_in = nc.dram_tensor("b1", [16, 1], dt.float32, kind="ExternalInput")
    w2_in = nc.dram_tensor("w2", [16, 1], dt.float32, kind="ExternalInput")
    b2_in = nc.dram_tensor("b2t", [1, 1], dt.float32, kind="ExternalInput")
    outp = nc.dram_tensor("out", [PCH * P, 1], dt.float32, kind="ExternalOutput")

    g_shardA = nc.dram_tensor("g_shardA", [HSA, H], dt.float16)
    g_shardB = nc.dram_tensor("g_shardB", [HSB, H], dt.float16)
    g_fullA = nc.dram_tensor("g_fullA", [NC * HSA, H], dt.float16,
                             addr_space="Shared")
    g_fullB = nc.dram_tensor("g_fullB", [NC * HSB, H], dt.float16,
                             addr_space="Shared")
    e_shards = [
        nc.dram_tensor(f"e_shard{i}", [PSPLITS[i] * P, H], dt.float16)
        for i in range(2)
    ]
    e_fulls = [
        nc.dram_tensor(f"e_full{i}", [NC * PSPLITS[i] * P, H], dt.float16,
                       addr_space="Shared")
        for i in range(2)
    ]

    g_tabs = [
        g_fullA[:, :].rearrange("(r two) f -> r (two f)", two=2),
        g_fullB[:, :].rearrange("(r two) f -> r (two f)", two=2),
    ]
    e_tabs = [
        ef[:, :].rearrange("(r two) f -> r (two f)", two=2) for ef in e_fulls
    ]

    def erot(g):
        r = (g + g // 4) % NBUCKET
        return [(r + j) % NBUCKET for j in range(NBUCKET)]

    def prot(pg):
        return [(pg + j) % PNBUCKET for j in range(PNBUCKET)]

    ngr = esched["ngroups"]
    e_entries = [(g, erot(g), [("eR", g)]) for g in range(ngr)]
    ephase = [g * NBUCKET + b for (g, bs, _) in e_entries for b in bs]
    eflags = _emit_flags(esched, [ephase])
    # pair: one PSUM accumulation phase per e-AG chunk; chunk A is copied
    # into xs, later chunks are added (so chunk-k gathers run while the
    # next chunk's AllGather is still in flight)
    p_entries = []
    for ph in range(2):
        for pg in range(psched["ngroups"]):
            bs = [ph * 2 + (pg & 1), ph * 2 + 1 - (pg & 1)]
            p_entries.append((pg, bs, [(pg, ph)]))
    pphases = [
        [pg * PNBUCKET + b for (pg, bs, _) in p_entries for b in bs
         if b // 2 == ph]
        for ph in range(2)
    ]
    pflags = _emit_flags(psched, pphases)

    with tile.TileContext(nc) as tc:
        nc.gpsimd.load_library(library_config.mlp)

        with (
            tc.tile_pool(name="const", bufs=1) as cpool,
            tc.tile_pool(name="dinvp", bufs=1) as dpool,
            tc.tile_pool(name="gres", bufs=1) as gpool,
        ):
            wc_sb = cpool.tile([P, 2, H], dt.float16)
            nc.sync.dma_start(wc_sb[:], wc_in[:, :, :])
            bconvb = cpool.tile([P, H], dt.float32)
            nc.sync.dma_start(bconvb[:], bconvb_in[:, :])
            ident = cpool.tile([P, P], dt.float16)
            nc.sync.dma_start(ident[:], ident_in[:, :])
            iota_sb = cpool.tile([P, P], dt.int16)
            nc.sync.dma_start(iota_sb[:], iota_in[:, :])
            w1_sb = cpool.tile([P, 16], dt.float16)
            nc.sync.dma_start(w1_sb[:], w1_in[:, :])
            b1_sb = cpool.tile([16, 1], dt.float32)
            nc.sync.dma_start(b1_sb[:], b1_in[:, :])
            w2_sb = cpool.tile([16, 1], dt.float32)
            nc.sync.dma_start(w2_sb[:], w2_in[:, :])
            b2_sb = cpool.tile([1, 1], dt.float32)
            nc.sync.dma_start(b2_sb[:], b2_in[:, :])
            deg_sb = dpool.tile([P, TILES], dt.float32)
            nc.sync.dma_start(deg_sb[:], deg_in[:, :])
            sq = dpool.tile([P, TILES], dt.float32)
            nc.scalar.activation(sq[:], deg_sb[:], mybir.ActivationFunctionType.Sqrt)
            dinv = dpool.tile([P, TILES], dt.float32)
            nc.vector.reciprocal(dinv[:], sq[:])

            g_sb = gpool.tile([P, TILES, H], dt.float16)
            g2_sb = gpool.tile([P, TILES, H], dt.float16)
            e_sb = g_sb  # g is dead once G2 is built; reuse its SBUF for e

            # ---------------- phase A: g = (x @ W) * dinv ----------------
            with (
                tc.tile_pool(name="xtp", bufs=2) as xtp,
                tc.tile_pool(name="hps", bufs=4, space="PSUM") as hps,
            ):
                blocks = [(0, 16), (16, 32), (32, 48),
                          (48, 64), (64, 81), (81, TILES)]
                for bi, (t0, t1) in enumerate(blocks):
                    xt_sb = xtp.tile([P, 2, (t1 - t0) * P], dt.float16, tag="xt")
                    eng = nc.sync if bi % 2 == 0 else nc.scalar
                    eng.dma_start(xt_sb[:], xt_in[:, :, t0 * P: t1 * P])
                    for t in range(t0, t1):
                        h_ps = hps.tile([P, H], dt.float32)
                        for k in range(2):
                            nc.tensor.matmul(
                                h_ps[:],
                                lhsT=xt_sb[:, k, (t - t0) * P:(t - t0 + 1) * P],
                                rhs=wc_sb[:, k, :],
                                start=(k == 0), stop=(k == 1),
                            )
                        nc.vector.tensor_scalar(
                            g_sb[:, t, :], h_ps[:], dinv[:, t:t + 1], None,
                            mybir.AluOpType.mult,
                        )
                    # write this block's g slice out immediately so the
                    # chunk AllGather can trigger as soon as its tiles land
                    shard, base = ((g_shardA, 0) if t1 <= SPLITS[0]
                                   else (g_shardB, SPLITS[0]))
                    weng = nc.scalar if bi % 2 == 0 else nc.sync
                    weng.dma_start(
                        shard[(t0 - base) * P:(t1 - base) * P, :].rearrange(
                            "(t p) f -> p t f", p=P),
                        g_sb[:, t0:t1, :],
                    )

                    if t1 == SPLITS[0]:
                        nc.gpsimd.collective_compute(
                            "AllGather", mybir.AluOpType.bypass,
                            replica_groups=[list(range(NC))],
                            ins=[g_shardA[:, :].opt()],
                            outs=[g_fullA[:, :].opt()],
                        )
                nc.gpsimd.collective_compute(
                    "AllGather", mybir.AluOpType.bypass,
                    replica_groups=[list(range(NC))],
                    ins=[g_shardB[:, :].opt()],
                    outs=[g_fullB[:, :].opt()],
                )
                # index/loc tables load behind phase A's x blocks; the first
                # gather needs them only once the g AllGather lands
                eidx_sb = cpool.tile([P, esched["totidx"] // 16], dt.int16)
                nc.scalar.dma_start(eidx_sb[:], eidx_in[:, :])
                pidx_sb = cpool.tile([P, psched["totidx"] // 16], dt.int16)
                nc.scalar.dma_start(pidx_sb[:], pidx_in[:, :])
                eloc_sb = cpool.tile([P, esched["totunits"]], dt.int16)
                nc.scalar.dma_start(eloc_sb[:], eloc_in[:, :])
                ploc_sb = cpool.tile([P, psched["totunits"]], dt.int16)
                nc.scalar.dma_start(ploc_sb[:], ploc_in[:, :])
                # G2 = g*dinv + b_conv (consume becomes acc*dinv + G2);
                # computed under the AllGathers, off the critical path
                for t in range(TILES):
                    nc.vector.tensor_scalar(
                        g2_sb[:, t, :], g_sb[:, t, :], dinv[:, t:t + 1],
                        None, mybir.AluOpType.mult,
                    )
                    nc.vector.tensor_tensor(
                        g2_sb[:, t, :], g2_sb[:, t, :], bconvb[:],
                        mybir.AluOpType.add,
                    )

            # ---------------- phase C: aggregate per dst tile ----------------
            EMSG_BUFS = 2
            with (
                tc.tile_pool(name="emsg", bufs=EMSG_BUFS) as msgp,
                tc.tile_pool(name="eoh", bufs=2) as ohp,
                tc.tile_pool(name="epost", bufs=4) as postp,
                tc.tile_pool(name="pmsg", bufs=2) as pmsgp,
                tc.tile_pool(name="poh", bufs=2) as pohp,
                tc.tile_pool(name="pxs", bufs=1) as pxsp,
            ):
                xs_sb = pxsp.tile([P, psched["ntiles"], H], dt.float16)
                ngroups_e = esched["ngroups"]
                ngroups_p = psched["ngroups"]

                def emit_gather(sched_, idx_sb, tabs, mp, w, prefix):
                    nb = sched_["nbucket"]
                    b = w % nb
                    K = int(sched_["K"][w])
                    ioff = int(sched_["woff"][w]) * P
                    msg = mp.tile([P, K, P], dt.float16, tag=f"{prefix}m{b}",
                                  name=f"{prefix}msg_w{w}")
                    # one gather per window: the SWDGE pipeline rate is
                    # insensitive to call size, so fewer calls = fewer FIFO
                    # slots and semaphore round-trips
                    halves = [(0, K)]
                    for (k0, k1) in halves:
                        if k0 == k1:
                            continue
                        hn = (k1 - k0) * P
                        hoff = ioff + k0 * P
                        nc.gpsimd.dma_gather(
                            msg[:, k0:k1, :], tabs[b // 2],
                            idx_sb[:, hoff // 16:(hoff + hn) // 16],
                            hn, hn, P, single_packet=False, queue_num=0)
                    return msg

                def emit_ohgen(sched_, loc_sb, op_, w, prefix):
                    # one-hot built on-device: iota is_equal the per-unit
                    # column table (255 => all-zero row, no contribution)
                    b = w % sched_["nbucket"]
                    uoff = int(sched_["uoff"][w])
                    nu = len(sched_["units"][w])
                    if nu == 0:
                        return None
                    oh = op_.tile([P, nu, P], dt.float8e4,
                                  tag=f"{prefix}o{b}" if prefix == "e"
                                  else f"po{b & 1}",
                                  name=f"{prefix}oh_w{w}")
                    nc.vector.tensor_tensor(
                        oh[:],
                        iota_sb[:].unsqueeze(1).broadcast_to([P, nu, P]),
                        loc_sb[:, uoff:uoff + nu].unsqueeze(2).broadcast_to(
                            [P, nu, P]),
                        mybir.AluOpType.is_equal,
                    )
                    return oh

                def emit_mms(sched_, flags, ap, acc_tiles, w, prefix,
                             msg, oh):
                    nb = sched_["nbucket"]
                    g = w // nb
                    b = w % nb
                    par = b & 1
                    for i, (ci, tt) in enumerate(sched_["units"][w]):
                        t = g * sched_["group_sz"] + tt
                        if t not in acc_tiles:
                            acc_tiles[t] = ap.tile(
                                [P, H], dt.float32,
                                tag=f"a{t % sched_['group_sz']}",
                                name=f"{prefix}acc_t{t}_w{w}")
                        st, sp = flags[(w, i)]
                        nc.tensor.matmul(
                            acc_tiles[t][:],
                            lhsT=oh[:, i, :],
                            rhs=msg[:, ci, par * H:(par + 1) * H],
                            start=st, stop=sp,
                        )

                def consume_edge(t, a):
                    # e = leaky(acc*dinv + G2); all-DVE to avoid cross-engine
                    # sem round-trips inside the serialized vector FIFO
                    s1 = postp.tile([P, H], dt.float32, tag="e1", name=f"e1_{t}")
                    nc.vector.tensor_scalar(
                        s1[:], a[:], dinv[:, t:t + 1], None, mybir.AluOpType.mult)
                    nc.vector.tensor_tensor(
                        s1[:], s1[:], g2_sb[:, t, :], mybir.AluOpType.add)
                    m = postp.tile([P, H], dt.float32, tag="m", name=f"m_{t}")
                    nc.vector.tensor_scalar(
                        m[:], s1[:], NEG, None, mybir.AluOpType.mult)
                    nc.vector.tensor_tensor(
                        e_sb[:, t, :], s1[:], m[:], mybir.AluOpType.max)

                with tc.tile_pool(name="eacc", bufs=1, space="PSUM") as accp:
                    # unified emission: pair chunk-k entries are interleaved
                    # into the edge stream as soon as e-AG chunk k has fired,
                    # so the SWDGE sequencer (the kernel's bottleneck) never
                    # idles behind a collective. The last e-AG (chunk 2) is
                    # triggered only after pair chunk-1 gathers are queued.
                    ekind = {
                        "sched": esched, "idx": eidx_sb, "loc": eloc_sb,
                        "tabs": g_tabs, "flags": eflags, "msgp": msgp,
                        "ohp": ohp, "nb": NBUCKET, "prefix": "e",
                    }
                    pkind = {
                        "sched": psched, "idx": pidx_sb, "loc": ploc_sb,
                        "tabs": e_tabs, "flags": pflags, "msgp": pmsgp,
                        "ohp": pohp, "nb": PNBUCKET, "prefix": "p",
                    }
                    p_by_phase = [[], []]
                    for ent in p_entries:
                        p_by_phase[ent[1][0] // 2].append(
                            (pkind, ent[0], ent[1],
                             [("p",) + c for c in ent[2]]))
                    e_seq = [(ekind,) + ent for ent in e_entries]
                    seq = (e_seq + [None] + p_by_phase[0] + p_by_phase[1])

                    def emit_e_consume(cg):
                        acc_c = group_acc.pop(("e", cg, 1))
                        t0 = cg * GROUP
                        t1 = min(t0 + GROUP, TILES)
                        for t in range(t0, t1):
                            consume_edge(t, acc_c.pop(t)[:])
                        # stream this group's e rows out right away; e-AG
                        # chunks 0/1 fire as soon as their last group lands
                        pb = np.concatenate([[0], np.cumsum(PSPLITS)])
                        ci_ = int(np.searchsorted(pb, t0, side="right")) - 1
                        base = int(pb[ci_])
                        nc.sync.dma_start(
                            e_shards[ci_][(t0 - base) * P:(t1 - base) * P, :]
                            .rearrange("(t p) f -> p t f", p=P),
                            e_sb[:, t0:t1, :],
                        )
                        if t1 == base + PSPLITS[ci_] and ci_ < 1:
                            nc.gpsimd.collective_compute(
                                "AllGather", mybir.AluOpType.bypass,
                                replica_groups=[list(range(NC))],
                                ins=[e_shards[ci_][:, :].opt()],
                                outs=[e_fulls[ci_][:, :].opt()],
                            )

                    def gen_ohs(ent):
                        k = ent[0]
                        g, bs = ent[1], ent[2]
                        for b in bs:
                            ohs[(k["prefix"], g * k["nb"] + b)] = emit_ohgen(
                                k["sched"], k["loc"], k["ohp"],
                                g * k["nb"] + b, k["prefix"])

                    ohs = {}
                    group_acc = {}
                    real = [s for s in seq if s is not None]
                    gen_ohs(real[0])
                    ri = 0
                    for ent in seq:
                        if ent is None:
                            nc.gpsimd.collective_compute(
                                "AllGather", mybir.AluOpType.bypass,
                                replica_groups=[list(range(NC))],
                                ins=[e_shards[1][:, :].opt()],
                                outs=[e_fulls[1][:, :].opt()],
                            )
                            continue
                        k, g, bs, consume = ent
                        pref, nb = k["prefix"], k["nb"]
                        if pref == "e":
                            acc_tiles = group_acc.setdefault(
                                ("e", g, 1), {})
                        else:
                            acc_tiles = group_acc.setdefault(
                                ("p", g, bs[0] // 2), {})
                        msgs = {}
                        for b in bs:
                            msgs[b] = emit_gather(k["sched"], k["idx"],
                                                  k["tabs"], k["msgp"],
                                                  g * nb + b, pref)
                        ri += 1
                        if ri < len(real):
                            gen_ohs(real[ri])
                        for b in bs:
                            oh = ohs.pop((pref, g * nb + b))
                            if oh is not None:
                                emit_mms(k["sched"], k["flags"], accp,
                                         acc_tiles, g * nb + b, pref,
                                         msgs[b], oh)
                        for c in consume:
                            if c[0] == "eR":
                                emit_e_consume(c[1])
                            else:
                                _, cg, ph = c
                                acc_c = group_acc.pop(("p", cg, ph))
                                for t in range(cg * PGROUP,
                                               (cg + 1) * PGROUP):
                                    if ph == 0:
                                        nc.vector.tensor_copy(
                                            xs_sb[:, t, :], acc_c.pop(t)[:])
                                    else:
                                        nc.vector.tensor_tensor(
                                            xs_sb[:, t, :], xs_sb[:, t, :],
                                            acc_c.pop(t)[:],
                                            mybir.AluOpType.add)

                # ---------------- phase D: pair MLP head ----------------
                with (
                    tc.tile_pool(name="ptps", bufs=4, space="PSUM") as ptps,
                    tc.tile_pool(name="pzps", bufs=2, space="PSUM") as pzps,
                    tc.tile_pool(name="pops", bufs=2, space="PSUM") as pops,
                    tc.tile_pool(name="psb", bufs=2) as psbp,
                ):
                    for k in range(PCH):
                        xt_ps = ptps.tile([P, P], dt.float16)
                        nc.tensor.transpose(xt_ps[0:H, :], xs_sb[:, k, :], ident[:])
                        nc.tensor.transpose(xt_ps[H:P, :], xs_sb[:, PCH + k, :],
                                            ident[:])
                        xijt = psbp.tile([P, P], dt.float16, tag="xijt")
                        nc.vector.tensor_copy(xijt[:], xt_ps[:])
                        z_ps = pzps.tile([16, P], dt.float32)
                        nc.tensor.matmul(z_ps[:], lhsT=w1_sb[:], rhs=xijt[:],
                                         start=True, stop=True)
                        zb = psbp.tile([16, P], dt.float32, tag="zb")
                        nc.vector.tensor_scalar(
                            zb[:], z_ps[:], b1_sb[:, 0:1], None, mybir.AluOpType.add)
                        m2 = psbp.tile([16, P], dt.float32, tag="m2")
                        nc.vector.tensor_scalar(
                            m2[:], zb[:], NEG, None, mybir.AluOpType.mult)
                        z2 = psbp.tile([16, P], dt.float32, tag="z2")
                        nc.vector.tensor_tensor(z2[:], zb[:], m2[:],
                                                mybir.AluOpType.max)
                        o_ps = pops.tile([1, P], dt.float32)
                        nc.tensor.matmul(o_ps[:], lhsT=w2_sb[:], rhs=z2[:],
                                         start=True, stop=True)
                        osb = psbp.tile([1, P], dt.float32, tag="osb")
                        nc.scalar.activation(
                            osb[:], o_ps[:], mybir.ActivationFunctionType.Sigmoid,
                            bias=b2_sb[:, 0:1], scale=1.0)
                        nc.sync.dma_start(
                            outp[k * P:(k + 1) * P, :].rearrange("r one -> one r"),
                            osb[0:1, :])

    # align each gather's SWDGE queue with its Tile-assigned DMA lane so
    # semaphore<->queue locking stays consistent (4-way parallel desc gen)
    for blk in nc.m.functions[0].blocks:
        for inst in blk.instructions:
            if isinstance(inst, mybir.InstDMAGatherAnt):
                si = inst.sync_info
                for u in (si.on_update if si else []):
                    mm = re.match(r"DMASW(\d+)_", u.ant_name or "")
                    if mm:
                        inst.queue_num = int(mm.group(1)) % 4
                        break

    nc.compile()
    return nc


def kernel(**inputs) -> np.ndarray:
    in_maps, sched = _prep(inputs)
    nc = _build(sched)
    res = run_bass_kernel_spmd(nc, in_maps, list(range(NC)))
    out = np.concatenate([res.results[c]["out"] for c in range(NC)], axis=0)
    return out.astype(np.float32)
